# revision 2
# baseline (speedup 1.0000x reference)
"""Cosine-similarity multi-head attention on 8 Trainium2 NeuronCores.

Sharding: tensor-parallel over (batch, head-group). Core c (c = b*4 + hg)
computes heads [4*hg, 4*hg+4) of batch b for ALL 2048 query tokens, then a
partial output projection over its 256 inner features.  The host sums the 4
partial outputs per batch and adds b_out (the "all-reduce" of the hint, done
during the host-side gather).  No K/V duplication: each projection row is
computed exactly once across the machine.

Per-core layouts:
  - xt   [128, 8, 2048]  x[b]^T, feature-chunked (bf16)
  - qnT/knT [128, 2, 2048] Q^T/K^T: chunk m holds local heads 2m (parts 0:64)
    and 2m+1 (parts 64:128); normalized in place (bf16)
  - av   [128, 16, 4, 128] V token-major per (key-chunk, head); cols 0:64 are
    a ones block so each A@V matmul also accumulates softmax denominators
  - softmax: no max-subtraction (|logits| <= 10, exp safe in f32)
  - exp is split across engines: Act computes exact exp; the DVE computes a
    Schraudolph bitcast exp (scale+bias -> int16 convert = bf16 bits) for a
    share of the keys.  Softmax self-normalization + averaging over many keys
    keeps the ~3% sawtooth error far below the output tolerance.
  - norm factors are broadcast across partitions with small K=128 PE matmuls
    against a 0/1 head-selector matrix (hsq) -- no DRAM round trips.
  - out-projection runs as a final phase reusing the attention PSUM rings;
    partial outputs leave as bf16 (summed in f32 on the host).
"""

import numpy as np

B, N, DIM, H, DH = 2, 2048, 1024, 16, 64
INNER = H * DH
P = 128
KC = DIM // P        # 8 contraction chunks of the model dim
JC = N // P          # 16 key-token chunks of 128
QB = 4               # query blocks of 512
NQ = N // QB         # 512
HL = 4               # heads per core
M = 2                # feature chunks per core (4 heads * 64 = 256)
MAX_LOG_SCALE = float(np.log(1.0 / 0.01))

# Schraudolph fast-exp constants (int16 convert writes bf16 bit pattern)
EXP_A16 = (2.0 ** 23 / np.log(2.0)) / 65536.0
EXP_B16 = (127.0 * 2.0 ** 23 - 360777.0) / 65536.0 + 0.5

ACT_FULL = 4         # every ACT_FULL-th key chunk: Act computes both heads
WARMUP = 34          # junk N=128 matmuls to warm the PE clock during DMA

_CACHE = {}


def _build():
    if "nc" in _CACHE:
        return _CACHE["nc"]
    import concourse.bass as bass
    import concourse.bacc as bacc
    import concourse.mybir as mybir
    import concourse.tile as tile

    f32 = mybir.dt.float32
    i16 = mybir.dt.int16
    bf16 = mybir.dt.bfloat16
    AF = mybir.ActivationFunctionType
    OP = mybir.AluOpType

    nc = bacc.Bacc("TRN2", target_bir_lowering=False)

    xTb = nc.declare_dram_parameter("xTb", [P, KC, N], bf16, isOutput=False)
    wqb = nc.declare_dram_parameter("wqb", [P, KC, M, P], bf16, isOutput=False)
    wkb = nc.declare_dram_parameter("wkb", [P, KC, M, P], bf16, isOutput=False)
    wvb = nc.declare_dram_parameter("wvb", [P, KC, M * P], bf16, isOutput=False)
    wob = nc.declare_dram_parameter("wob", [P, M, KC, P], bf16, isOutput=False)
    hsq = nc.declare_dram_parameter("hsq", [P, M, P], bf16, isOutput=False)
    sclq2 = nc.declare_dram_parameter("sclq2", [P, M], f32, isOutput=False)
    outT = nc.declare_dram_parameter("outT", [DIM, N], bf16, isOutput=True)

    with tile.TileContext(nc) as tc:
        with (
            tc.tile_pool(name="persist", bufs=1) as pp,
            tc.tile_pool(name="work", bufs=2) as pa,
            tc.tile_pool(name="pout", bufs=4) as pout,
            tc.tile_pool(name="ps", bufs=2, space="PSUM") as ps,
        ):
            xt = pp.tile([P, KC, N], bf16, tag="xt")
            qnT = pp.tile([P, M, N], bf16, tag="qnT")
            knT = pp.tile([P, M, N], bf16, tag="knT")
            av = pp.tile([P, JC, HL, P], bf16, tag="av")
            onT = pp.tile([P, M, N], bf16, tag="onT")
            wq_sb = pp.tile([P, KC, M, P], bf16, tag="wq")
            wk_sb = pp.tile([P, KC, M, P], bf16, tag="wk")
            wv_sb = pp.tile([P, KC, M * P], bf16, tag="wv")
            wo_sb = pp.tile([P, M, KC, P], bf16, tag="wo")
            hsq_sb = pp.tile([P, M, P], bf16, tag="hsq")
            scl_sb = pp.tile([P, M], f32, tag="scl")
            zero_b = pp.tile([P, 1], f32, tag="zerob")

            # ---- input DMA: three queues; x + the weights needed first ----
            nc.sync.dma_start(out=hsq_sb[:], in_=hsq[:])
            nc.sync.dma_start(out=scl_sb[:], in_=sclq2[:])
            nc.scalar.dma_start(out=wk_sb[:], in_=wkb[:])
            for tq in range(QB):
                qs = slice(tq * NQ, (tq + 1) * NQ)
                nc.sync.dma_start(out=xt[:, 0:4, qs], in_=xTb[:, 0:4, qs])
                nc.gpsimd.dma_start(out=xt[:, 4:KC, qs], in_=xTb[:, 4:KC, qs])
            nc.scalar.dma_start(out=wq_sb[:], in_=wqb[:])
            nc.sync.dma_start(out=wv_sb[:], in_=wvb[:])
            nc.gpsimd.dma_start(out=wo_sb[:], in_=wob[:])

            nc.vector.memset(zero_b[:], 0.0)
            nc.vector.memset(av[:, :, :, 0:DH], 1.0)

            # ---- PE warmup: junk matmuls during the initial DMA so the
            # clock gate is released before real work arrives ----
            warm = ps.tile([P, 2, NQ], f32, tag="avp", name="warm")
            for _ in range(WARMUP):
                nc.tensor.matmul(warm[:, 0, 0:P], hsq_sb[:, 0, :],
                                 hsq_sb[:, 0, :], start=True, stop=True)

            # ---------------- Phase A: K then Q projections + norms --------
            def factor_apply(nT, sqs_list, qb):
                qs = slice(qb * NQ, (qb + 1) * NQ)
                for m in range(M):
                    sqf = pa.tile([P, NQ], f32, tag="sqf")
                    nc.vector.reciprocal_approx_fast(
                        out=sqf[:], in_=sqs_list[m][:])
                    nc.gpsimd.tensor_mul(nT[:, m, qs], sqf[:], nT[:, m, qs])

            def emit_norm(pend_sq, q_scale):
                psq, pm, plist = pend_sq
                pn = ps.tile([P, NQ], f32, tag="avp", name="pn")
                nc.tensor.matmul(pn[:], hsq_sb[:, pm, :], psq[:],
                                 start=True, stop=True)
                sqs = pa.tile([P, NQ], f32, tag="sqs", bufs=4)
                nc.scalar.activation(
                    sqs[:], pn[:], AF.Sqrt, bias=zero_b[:],
                    scale=scl_sb[:, pm : pm + 1] if q_scale else 1.0)
                plist.append(sqs)

            def proj_side(w_sb, nT, q_scale, evac_eng):
                # norm matmul for each (qb, m) emitted one step late so the
                # PE never waits in-order on the Act square chain
                pend = None
                pend_sq = None
                for qb in range(QB):
                    qs = slice(qb * NQ, (qb + 1) * NQ)
                    sqs_list = []
                    for m in range(M):
                        pq = ps.tile([P, NQ], f32, tag="sps", name="pq")
                        for kc in range(KC):
                            nc.tensor.matmul(pq[:], w_sb[:, kc, m, :],
                                             xt[:, kc, qs],
                                             start=(kc == 0),
                                             stop=(kc == KC - 1))
                        if evac_eng == 0:
                            nc.vector.tensor_copy(nT[:, m, qs], pq[:])
                        else:
                            nc.scalar.copy(nT[:, m, qs], pq[:])
                        sq = pa.tile([P, NQ], bf16, tag="sq")
                        nc.scalar.activation(sq[:], pq[:], AF.Square,
                                             bias=zero_b[:])
                        if pend_sq is not None:
                            emit_norm(pend_sq, q_scale)
                        pend_sq = (sq, m, sqs_list)
                    if pend is not None:
                        factor_apply(nT, *pend)
                    pend = (sqs_list, qb)
                emit_norm(pend_sq, q_scale)
                factor_apply(nT, *pend)

            proj_side(wk_sb, knT, False, 1)  # K first: evacuate via Act
            proj_side(wq_sb, qnT, True, 0)   # Q: temp folded into sqrt scale

            # ------------- Phase B: attention (+ V weave in sweep 0) -------
            def emit_v(jc):
                pv = ps.tile([P, M * P], f32, tag="sps", name="pv")
                for kc in range(KC):
                    nc.tensor.matmul(pv[:], xt[:, kc, jc * P : (jc + 1) * P],
                                     wv_sb[:, kc, :],
                                     start=(kc == 0), stop=(kc == KC - 1))
                nc.vector.tensor_copy(
                    av[:, jc, :, DH:P],
                    pv[:].rearrange("p (h d) -> p h d", d=DH))

            pend = [None]

            def flush_pend():
                if pend[0] is None:
                    return
                pet, pkc, pavp, pqb, ppr = pend[0]
                pend[0] = None
                nc.tensor.matmul(pavp[:, 0, :], av[:, pkc, 2 * ppr, :],
                                 pet[:, 0, :],
                                 start=(pkc == 0), stop=(pkc == JC - 1))
                nc.tensor.matmul(pavp[:, 1, :], av[:, pkc, 2 * ppr + 1, :],
                                 pet[:, 1, :],
                                 start=(pkc == 0), stop=(pkc == JC - 1))
                if pkc == JC - 1:
                    # denominators arrive replicated on partitions 0:64 (the
                    # ones block); one fused reciprocal covers both heads
                    pqs = slice(pqb * NQ, (pqb + 1) * NQ)
                    rec = pa.tile([DH, 2, NQ], f32, tag="dn", name="rec")
                    nc.vector.reciprocal_approx_fast(
                        out=rec[:], in_=pavp[0:DH, :, :])
                    nc.vector.tensor_mul(onT[0:DH, ppr, pqs],
                                         pavp[DH:P, 0, :], rec[:, 0, :])
                    nc.vector.tensor_mul(onT[DH:P, ppr, pqs],
                                         pavp[DH:P, 1, :], rec[:, 1, :])

            for qb in range(QB):
                qs = slice(qb * NQ, (qb + 1) * NQ)
                for pr in range(M):       # head pair (2pr, 2pr+1)
                    avp = ps.tile([P, 2, NQ], f32, tag="avp", name="avp")
                    for kc in range(JC):
                        ks = slice(kc * P, (kc + 1) * P)
                        if qb == 0 and pr == 0:
                            emit_v(kc)    # weave V projection into sweep 0
                        sp = ps.tile([P, 2, NQ], f32, tag="sps", name="sp")
                        nc.tensor.matmul(sp[:, 0, :], knT[0:64, pr, ks],
                                         qnT[0:64, pr, qs],
                                         start=True, stop=True)
                        nc.tensor.matmul(sp[:, 1, :], knT[64:P, pr, ks],
                                         qnT[64:P, pr, qs],
                                         start=True, stop=True)
                        # A@V runs one kc behind (across sweep boundaries
                        # too) so the PE never waits in-order on the exp it
                        # just requested
                        flush_pend()
                        et = pa.tile([P, 2, NQ], bf16, tag="et", bufs=4)
                        if kc % ACT_FULL == ACT_FULL - 1:
                            nc.scalar.activation(et[:], sp[:], AF.Exp,
                                                 bias=zero_b[:])
                        else:
                            nc.scalar.activation(et[:, 0, :], sp[:, 0, :],
                                                 AF.Exp, bias=zero_b[:])
                            nc.vector.tensor_scalar(
                                out=et[:, 1, :].bitcast(i16),
                                in0=sp[:, 1, :],
                                scalar1=EXP_A16, scalar2=EXP_B16,
                                op0=OP.mult, op1=OP.add)
                        pend[0] = (et, kc, avp, qb, pr)
            flush_pend()

            # ------------- Phase C: output projection ----------------------
            for qb in range(QB):
                qs = slice(qb * NQ, (qb + 1) * NQ)
                for mo in range(KC):
                    idx = qb * KC + mo
                    cp = ps.tile([P, NQ], f32,
                                 tag=("sps" if idx % 2 == 0 else "avp"),
                                 name="cp")
                    for g in range(M):
                        nc.tensor.matmul(cp[:], wo_sb[:, g, mo, :],
                                         onT[:, g, qs],
                                         start=(g == 0), stop=(g == M - 1))
                    oc = pout.tile([P, NQ], bf16, tag="ot", name="oc")
                    if idx % 2 == 0:
                        nc.vector.tensor_copy(oc[:], cp[:])
                    else:
                        nc.scalar.copy(oc[:], cp[:])
                    nc.sync.dma_start(out=outT[mo * P : (mo + 1) * P, qs],
                                      in_=oc[:])

    nc.compile()
    _CACHE["nc"] = nc
    return nc


def run(inputs, trace=False):
    import ml_dtypes
    from concourse.bass_utils import run_bass_kernel_spmd

    x = np.asarray(inputs["x"], np.float32)
    w_qkv = np.asarray(inputs["w_qkv"], np.float32)
    w_out = np.asarray(inputs["w_out"], np.float32)
    b_out = np.asarray(inputs["b_out"], np.float32)
    logit_scale = np.asarray(inputs["logit_scale"], np.float32)

    nc = _build()
    bf = ml_dtypes.bfloat16

    scl = np.exp(np.minimum(logit_scale.reshape(H), MAX_LOG_SCALE))

    xTb = [np.ascontiguousarray(
        x[b].T.reshape(KC, P, N).transpose(1, 0, 2)).astype(bf)
        for b in range(B)]

    in_maps = []
    for c in range(8):
        b, hg = c // 4, c % 4
        cs = slice(hg * 256, (hg + 1) * 256)
        wq = np.ascontiguousarray(
            w_qkv[:, 0:INNER][:, cs].reshape(KC, P, M, P)
            .transpose(1, 0, 2, 3)).astype(bf)
        wk = np.ascontiguousarray(
            w_qkv[:, INNER:2 * INNER][:, cs].reshape(KC, P, M, P)
            .transpose(1, 0, 2, 3)).astype(bf)
        wv = np.ascontiguousarray(
            w_qkv[:, 2 * INNER:3 * INNER][:, cs].reshape(KC, P, M * P)
            .transpose(1, 0, 2)).astype(bf)
        wo = np.ascontiguousarray(
            w_out[cs, :].reshape(M, P, KC, P).transpose(1, 0, 2, 3)).astype(bf)
        hs = np.zeros((P, M, P), bf)
        hs[0:64, :, 0:64] = 1.0
        hs[64:P, :, 64:P] = 1.0
        sc2 = np.empty((P, M), np.float32)
        for m in range(M):
            sc2[0:64, m] = scl[4 * hg + 2 * m] ** -2.0
            sc2[64:P, m] = scl[4 * hg + 2 * m + 1] ** -2.0
        in_maps.append({
            "xTb": xTb[b], "wqb": wq, "wkb": wk, "wvb": wv, "wob": wo,
            "hsq": hs, "sclq2": sc2,
        })

    res = run_bass_kernel_spmd(nc, in_maps, list(range(8)), trace=trace)

    out = np.empty((B, N, DIM), np.float32)
    for b in range(B):
        acc = res.results[4 * b]["outT"].astype(np.float32)
        for hg in range(1, 4):
            acc = acc + res.results[4 * b + hg]["outT"].astype(np.float32)
        out[b] = acc.T + b_out
    return out, res


def kernel(**inputs):
    out, _ = run(inputs, trace=False)
    return out


# revision 11
# speedup vs baseline: 1.1118x; 1.1118x over previous
"""Cosine-similarity multi-head attention on 8 Trainium2 NeuronCores.

Sharding: tensor-parallel over (batch, head-group). Core c (c = b*4 + hg)
computes heads [4*hg, 4*hg+4) of batch b for ALL 2048 query tokens, then a
partial output projection over its 256 inner features.  The host sums the 4
partial outputs per batch and adds b_out (the "all-reduce" of the hint, done
during the host-side gather).  No K/V duplication: each projection row is
computed exactly once across the machine.

Per-core layouts:
  - xt   [128, 8, 2048]  x[b]^T, feature-chunked (bf16)
  - qnT/knT [128, 2, 2048] Q^T/K^T: chunk m holds local heads 2m (parts 0:64)
    and 2m+1 (parts 64:128); normalized in place (bf16)
  - av   [128, 16, 4, 128] V token-major per (key-chunk, head); cols 0:64 are
    a ones block so each A@V matmul also accumulates softmax denominators
  - softmax: no max-subtraction (|logits| <= 10, exp safe in f32)
  - exp is split across engines: Act computes exact exp; the DVE computes a
    Schraudolph bitcast exp (scale+bias -> int16 convert = bf16 bits) for a
    share of the keys.  Softmax self-normalization + averaging over many keys
    keeps the ~3% sawtooth error far below the output tolerance.
  - norm factors are broadcast across partitions with small K=128 PE matmuls
    against a 0/1 head-selector matrix (hsq) -- no DRAM round trips.
  - out-projection runs as a final phase reusing the attention PSUM rings;
    partial outputs leave as bf16 (summed in f32 on the host).
"""

import numpy as np

B, N, DIM, H, DH = 2, 2048, 1024, 16, 64
INNER = H * DH
P = 128
KC = DIM // P        # 8 contraction chunks of the model dim
JC = N // P          # 16 key-token chunks of 128
QB = 4               # query blocks of 512
NQ = N // QB         # 512
HL = 4               # heads per core
M = 2                # feature chunks per core (4 heads * 64 = 256)
MAX_LOG_SCALE = float(np.log(1.0 / 0.01))

# Schraudolph fast-exp constants (int16 convert writes bf16 bit pattern)
EXP_A16 = (2.0 ** 23 / np.log(2.0)) / 65536.0
EXP_B16 = (127.0 * 2.0 ** 23 - 360777.0) / 65536.0 + 0.5

ACT_FULL = 4         # every ACT_FULL-th key chunk: Act computes both heads
WARMUP = 34          # junk N=128 matmuls to warm the PE clock during DMA

_CACHE = {}


def _build():
    if "nc" in _CACHE:
        return _CACHE["nc"]
    import concourse.bass as bass
    import concourse.bacc as bacc
    import concourse.mybir as mybir
    import concourse.tile as tile

    f32 = mybir.dt.float32
    i16 = mybir.dt.int16
    bf16 = mybir.dt.bfloat16
    AF = mybir.ActivationFunctionType
    OP = mybir.AluOpType

    nc = bacc.Bacc("TRN2", target_bir_lowering=False)

    xTb = nc.declare_dram_parameter("xTb", [P, QB, KC, NQ], bf16,
                                    isOutput=False)
    wqb = nc.declare_dram_parameter("wqb", [P, KC, M, P], bf16, isOutput=False)
    wkb = nc.declare_dram_parameter("wkb", [P, KC, M, P], bf16, isOutput=False)
    wvb = nc.declare_dram_parameter("wvb", [P, KC, M * P], bf16, isOutput=False)
    wob = nc.declare_dram_parameter("wob", [P, M, KC, P], bf16, isOutput=False)
    hsq = nc.declare_dram_parameter("hsq", [P, M, P], bf16, isOutput=False)
    sclq2 = nc.declare_dram_parameter("sclq2", [P, M], f32, isOutput=False)
    outT = nc.declare_dram_parameter("outT", [DIM, N], bf16, isOutput=True)

    with tile.TileContext(nc) as tc:
        with (
            tc.tile_pool(name="persist", bufs=1) as pp,
            tc.tile_pool(name="work", bufs=2) as pa,
            tc.tile_pool(name="pout", bufs=4) as pout,
            tc.tile_pool(name="ps", bufs=2, space="PSUM") as ps,
        ):
            # one tile (and one DMA) per 512-token block so the first
            # projection matmul only waits on its own block's transfer
            xts = [pp.tile([P, KC, NQ], bf16, tag=f"xt{i}", name=f"xt{i}")
                   for i in range(QB)]
            wu = pp.tile([P, P], bf16, tag="wu")
            qnT = pp.tile([P, M, N], bf16, tag="qnT")
            knT = pp.tile([P, M, N], bf16, tag="knT")
            av = pp.tile([P, JC, HL, P], bf16, tag="av")
            onT = pp.tile([P, M, N], bf16, tag="onT")
            wq_sb = pp.tile([P, KC, M, P], bf16, tag="wq")
            wk_sb = pp.tile([P, KC, M, P], bf16, tag="wk")
            wv_sb = pp.tile([P, KC, M * P], bf16, tag="wv")
            wo_sb = pp.tile([P, M, KC, P], bf16, tag="wo")
            hsq_sb = pp.tile([P, M, P], bf16, tag="hsq")
            scl_sb = pp.tile([P, M], f32, tag="scl")
            zero_b = pp.tile([P, 1], f32, tag="zerob")

            # ---- input DMA: three queues; x + the weights needed first ----
            nc.sync.dma_start(out=hsq_sb[:], in_=hsq[:])
            nc.sync.dma_start(out=scl_sb[:], in_=sclq2[:])
            nc.scalar.dma_start(out=wk_sb[:], in_=wkb[:])
            nc.sync.dma_start(out=xts[0][:], in_=xTb[:, 0])
            nc.gpsimd.dma_start(out=xts[1][:], in_=xTb[:, 1])
            nc.sync.dma_start(out=xts[2][:], in_=xTb[:, 2])
            nc.gpsimd.dma_start(out=xts[3][:], in_=xTb[:, 3])
            nc.scalar.dma_start(out=wq_sb[:], in_=wqb[:])
            nc.sync.dma_start(out=wv_sb[:], in_=wvb[:])
            nc.gpsimd.dma_start(out=wo_sb[:], in_=wob[:])

            nc.vector.memset(zero_b[:], 0.0)
            nc.vector.memset(wu[:], 0.0)
            nc.vector.memset(av[:, :, :, 0:DH], 1.0)

            # ---- PE warmup: junk matmuls during the initial DMA so the
            # clock gate is released before real work arrives; operand is a
            # memset tile so nothing waits on DMA ----
            warm = ps.tile([P, 2, NQ], f32, tag="avp", name="warm")
            for _ in range(WARMUP):
                nc.tensor.matmul(warm[:, 0, 0:P], wu[:], wu[:],
                                 start=True, stop=True)

            # ---------------- Phase A: K then Q projections + norms --------
            def factor_apply(nT, sqs_list, qb):
                qs = slice(qb * NQ, (qb + 1) * NQ)
                for m in range(M):
                    sqf = pa.tile([P, NQ], f32, tag="sqf")
                    nc.vector.reciprocal_approx_fast(
                        out=sqf[:], in_=sqs_list[m][:])
                    nc.gpsimd.tensor_mul(nT[:, m, qs], sqf[:], nT[:, m, qs])

            def emit_norm(pend_sq, q_scale):
                psq, pm, plist = pend_sq
                pn = ps.tile([P, NQ], f32, tag="avp", name="pn")
                nc.tensor.matmul(pn[:], hsq_sb[:, pm, :], psq[:],
                                 start=True, stop=True)
                sqs = pa.tile([P, NQ], f32, tag="sqs", bufs=4)
                nc.scalar.activation(
                    sqs[:], pn[:], AF.Sqrt, bias=zero_b[:],
                    scale=scl_sb[:, pm : pm + 1] if q_scale else 1.0)
                plist.append(sqs)

            def proj_side(w_sb, nT, q_scale):
                # norm matmul for each (qb, m) emitted one step late so the
                # PE never waits in-order on the Act square chain
                pend = None
                pend_sq = None
                for qb in range(QB):
                    qs = slice(qb * NQ, (qb + 1) * NQ)
                    sqs_list = []
                    for m in range(M):
                        pq = ps.tile([P, NQ], f32, tag="sps", name="pq")
                        for kc in range(KC):
                            nc.tensor.matmul(pq[:], w_sb[:, kc, m, :],
                                             xts[qb][:, kc, :],
                                             start=(kc == 0),
                                             stop=(kc == KC - 1))
                        nc.vector.tensor_copy(nT[:, m, qs], pq[:])
                        sq = pa.tile([P, NQ], bf16, tag="sq")
                        nc.scalar.activation(sq[:], pq[:], AF.Square,
                                             bias=zero_b[:])
                        if pend_sq is not None:
                            emit_norm(pend_sq, q_scale)
                        pend_sq = (sq, m, sqs_list)
                    if pend is not None:
                        factor_apply(nT, *pend)
                    pend = (sqs_list, qb)
                emit_norm(pend_sq, q_scale)
                factor_apply(nT, *pend)

            proj_side(wk_sb, knT, False)  # K first (all keys needed first)
            proj_side(wq_sb, qnT, True)   # Q: temp folded into sqrt scale

            # ------------- Phase B: attention (+ V weave in sweep 0) -------
            def emit_v(jc):
                pv = ps.tile([P, M * P], f32, tag="sps", name="pv")
                xj = xts[jc // 4]
                js = slice((jc % 4) * P, (jc % 4) * P + P)
                for kc in range(KC):
                    nc.tensor.matmul(pv[:], xj[:, kc, js],
                                     wv_sb[:, kc, :],
                                     start=(kc == 0), stop=(kc == KC - 1))
                nc.vector.tensor_copy(
                    av[:, jc, :, DH:P],
                    pv[:].rearrange("p (h d) -> p h d", d=DH))

            pend = []

            def flush_pend():
                if not pend:
                    return
                pet, pkc, pavp, pqb, ppr = pend.pop(0)
                nc.tensor.matmul(pavp[:, 0, :], av[:, pkc, 2 * ppr, :],
                                 pet[:, 0, :],
                                 start=(pkc == 0), stop=(pkc == JC - 1))
                nc.tensor.matmul(pavp[:, 1, :], av[:, pkc, 2 * ppr + 1, :],
                                 pet[:, 1, :],
                                 start=(pkc == 0), stop=(pkc == JC - 1))
                if pkc == JC - 1:
                    # denominators arrive replicated on partitions 0:64 (the
                    # ones block); one fused reciprocal covers both heads
                    pqs = slice(pqb * NQ, (pqb + 1) * NQ)
                    rec = pa.tile([DH, 2, NQ], f32, tag="dn", name="rec")
                    nc.vector.reciprocal_approx_fast(
                        out=rec[:], in_=pavp[0:DH, :, :])
                    nc.vector.tensor_mul(onT[0:DH, ppr, pqs],
                                         pavp[DH:P, 0, :], rec[:, 0, :])
                    nc.vector.tensor_mul(onT[DH:P, ppr, pqs],
                                         pavp[DH:P, 1, :], rec[:, 1, :])

            for qb in range(QB):
                qs = slice(qb * NQ, (qb + 1) * NQ)
                for pr in range(M):       # head pair (2pr, 2pr+1)
                    avp = ps.tile([P, 2, NQ], f32, tag="avp", name="avp")
                    for kc in range(JC):
                        ks = slice(kc * P, (kc + 1) * P)
                        if qb == 0 and pr == 0:
                            emit_v(kc)    # weave V projection into sweep 0
                        sp = ps.tile([P, 2, NQ], f32, tag="sps", name="sp")
                        nc.tensor.matmul(sp[:, 0, :], knT[0:64, pr, ks],
                                         qnT[0:64, pr, qs],
                                         start=True, stop=True)
                        nc.tensor.matmul(sp[:, 1, :], knT[64:P, pr, ks],
                                         qnT[64:P, pr, qs],
                                         start=True, stop=True)
                        # A@V runs two kc behind (across sweep boundaries
                        # too) so the PE never waits in-order on exps still
                        # in flight on the Act/DVE queues
                        if len(pend) >= 2:
                            flush_pend()
                        et = pa.tile([P, 2, NQ], bf16, tag="et", bufs=5)
                        if kc % ACT_FULL == ACT_FULL - 1:
                            nc.scalar.activation(et[:], sp[:], AF.Exp,
                                                 bias=zero_b[:])
                        else:
                            nc.scalar.activation(et[:, 0, :], sp[:, 0, :],
                                                 AF.Exp, bias=zero_b[:])
                            nc.vector.tensor_scalar(
                                out=et[:, 1, :].bitcast(i16),
                                in0=sp[:, 1, :],
                                scalar1=EXP_A16, scalar2=EXP_B16,
                                op0=OP.mult, op1=OP.add)
                        pend.append((et, kc, avp, qb, pr))
            flush_pend()
            flush_pend()

            # ------------- Phase C: output projection ----------------------
            for qb in range(QB):
                qs = slice(qb * NQ, (qb + 1) * NQ)
                for mo in range(KC):
                    idx = qb * KC + mo
                    cp = ps.tile([P, NQ], f32,
                                 tag=("sps" if idx % 2 == 0 else "avp"),
                                 name="cp")
                    for g in range(M):
                        nc.tensor.matmul(cp[:], wo_sb[:, g, mo, :],
                                         onT[:, g, qs],
                                         start=(g == 0), stop=(g == M - 1))
                    oc = pout.tile([P, NQ], bf16, tag="ot", name="oc",
                                   bufs=6)
                    if idx % 2 == 0:
                        nc.vector.tensor_copy(oc[:], cp[:])
                    else:
                        nc.scalar.copy(oc[:], cp[:])
                    eng = nc.sync if idx % 2 == 0 else nc.gpsimd
                    eng.dma_start(out=outT[mo * P : (mo + 1) * P, qs],
                                  in_=oc[:])

    nc.compile()
    _CACHE["nc"] = nc
    return nc


def run(inputs, trace=False):
    import ml_dtypes
    from concourse.bass_utils import run_bass_kernel_spmd

    x = np.asarray(inputs["x"], np.float32)
    w_qkv = np.asarray(inputs["w_qkv"], np.float32)
    w_out = np.asarray(inputs["w_out"], np.float32)
    b_out = np.asarray(inputs["b_out"], np.float32)
    logit_scale = np.asarray(inputs["logit_scale"], np.float32)

    nc = _build()
    bf = ml_dtypes.bfloat16

    scl = np.exp(np.minimum(logit_scale.reshape(H), MAX_LOG_SCALE))

    # [P, QB, KC, NQ]: partition-major, then 512-token block, then dim chunk
    xTb = [np.ascontiguousarray(
        x[b].T.reshape(KC, P, QB, NQ).transpose(1, 2, 0, 3)).astype(bf)
        for b in range(B)]

    in_maps = []
    for c in range(8):
        b, hg = c // 4, c % 4
        cs = slice(hg * 256, (hg + 1) * 256)
        wq = np.ascontiguousarray(
            w_qkv[:, 0:INNER][:, cs].reshape(KC, P, M, P)
            .transpose(1, 0, 2, 3)).astype(bf)
        wk = np.ascontiguousarray(
            w_qkv[:, INNER:2 * INNER][:, cs].reshape(KC, P, M, P)
            .transpose(1, 0, 2, 3)).astype(bf)
        wv = np.ascontiguousarray(
            w_qkv[:, 2 * INNER:3 * INNER][:, cs].reshape(KC, P, M * P)
            .transpose(1, 0, 2)).astype(bf)
        wo = np.ascontiguousarray(
            w_out[cs, :].reshape(M, P, KC, P).transpose(1, 0, 2, 3)).astype(bf)
        hs = np.zeros((P, M, P), bf)
        hs[0:64, :, 0:64] = 1.0
        hs[64:P, :, 64:P] = 1.0
        sc2 = np.empty((P, M), np.float32)
        for m in range(M):
            sc2[0:64, m] = scl[4 * hg + 2 * m] ** -2.0
            sc2[64:P, m] = scl[4 * hg + 2 * m + 1] ** -2.0
        in_maps.append({
            "xTb": xTb[b], "wqb": wq, "wkb": wk, "wvb": wv, "wob": wo,
            "hsq": hs, "sclq2": sc2,
        })

    res = run_bass_kernel_spmd(nc, in_maps, list(range(8)), trace=trace)

    out = np.empty((B, N, DIM), np.float32)
    for b in range(B):
        acc = res.results[4 * b]["outT"].astype(np.float32)
        for hg in range(1, 4):
            acc = acc + res.results[4 * b + hg]["outT"].astype(np.float32)
        out[b] = acc.T + b_out
    return out, res


def kernel(**inputs):
    out, _ = run(inputs, trace=False)
    return out


# revision 17
# speedup vs baseline: 1.1547x; 1.0386x over previous
"""Cosine-similarity multi-head attention on 8 Trainium2 NeuronCores.

Sharding: tensor-parallel over (batch, head-group). Core c (c = b*4 + hg)
computes heads [4*hg, 4*hg+4) of batch b for ALL 2048 query tokens, then a
partial output projection over its 256 inner features.  The host sums the 4
partial outputs per batch and adds b_out (the "all-reduce" of the hint, done
during the host-side gather).  No K/V duplication: each projection row is
computed exactly once across the machine.

Per-core layouts:
  - xt   [128, 8, 2048]  x[b]^T, feature-chunked (bf16)
  - qnT/knT [128, 2, 2048] Q^T/K^T: chunk m holds local heads 2m (parts 0:64)
    and 2m+1 (parts 64:128); normalized in place (bf16)
  - av   [128, 16, 4, 128] V token-major per (key-chunk, head); cols 0:64 are
    a ones block so each A@V matmul also accumulates softmax denominators
  - softmax: no max-subtraction (|logits| <= 10, exp safe in f32)
  - exp is split across engines: Act computes exact exp; the DVE computes a
    Schraudolph bitcast exp (scale+bias -> int16 convert = bf16 bits) for a
    share of the keys.  Softmax self-normalization + averaging over many keys
    keeps the ~3% sawtooth error far below the output tolerance.
  - norm factors are broadcast across partitions with small K=128 PE matmuls
    against a 0/1 head-selector matrix (hsq) -- no DRAM round trips.
  - out-projection runs as a final phase reusing the attention PSUM rings;
    partial outputs leave as bf16 (summed in f32 on the host).
"""

import numpy as np

B, N, DIM, H, DH = 2, 2048, 1024, 16, 64
INNER = H * DH
P = 128
KC = DIM // P        # 8 contraction chunks of the model dim
JC = N // P          # 16 key-token chunks of 128
QB = 4               # query blocks of 512
NQ = N // QB         # 512
HL = 4               # heads per core
M = 2                # feature chunks per core (4 heads * 64 = 256)
MAX_LOG_SCALE = float(np.log(1.0 / 0.01))

# Schraudolph fast-exp constants (int16 convert writes bf16 bit pattern)
EXP_A16 = (2.0 ** 23 / np.log(2.0)) / 65536.0
EXP_B16 = (127.0 * 2.0 ** 23 - 360777.0) / 65536.0 + 0.5

ACT_FULL = 4         # every ACT_FULL-th key chunk: Act computes both heads
WARMUP = 34          # junk N=128 matmuls to warm the PE clock during DMA

_CACHE = {}


def _build():
    if "nc" in _CACHE:
        return _CACHE["nc"]
    import concourse.bass as bass
    import concourse.bacc as bacc
    import concourse.mybir as mybir
    import concourse.tile as tile

    f32 = mybir.dt.float32
    i16 = mybir.dt.int16
    bf16 = mybir.dt.bfloat16
    AF = mybir.ActivationFunctionType
    OP = mybir.AluOpType

    nc = bacc.Bacc("TRN2", target_bir_lowering=False)

    xTb = nc.declare_dram_parameter("xTb", [P, QB, KC, NQ], bf16,
                                    isOutput=False)
    wqb = nc.declare_dram_parameter("wqb", [P, KC, M, P], bf16, isOutput=False)
    wkb = nc.declare_dram_parameter("wkb", [P, KC, M, P], bf16, isOutput=False)
    wvb = nc.declare_dram_parameter("wvb", [P, KC, M * P], bf16, isOutput=False)
    wob = nc.declare_dram_parameter("wob", [P, M, KC, P], bf16, isOutput=False)
    hsq = nc.declare_dram_parameter("hsq", [P, M, P], bf16, isOutput=False)
    sclq2 = nc.declare_dram_parameter("sclq2", [P, M], f32, isOutput=False)
    outT = nc.declare_dram_parameter("outT", [DIM, N], bf16, isOutput=True)

    with tile.TileContext(nc) as tc:
        with (
            tc.tile_pool(name="persist", bufs=1) as pp,
            tc.tile_pool(name="work", bufs=2) as pa,
            tc.tile_pool(name="pout", bufs=4) as pout,
            tc.tile_pool(name="ps", bufs=2, space="PSUM") as ps,
        ):
            # one tile (and one DMA) per 512-token block so the first
            # projection matmul only waits on its own block's transfer
            xts = [pp.tile([P, KC, NQ], bf16, tag=f"xt{i}", name=f"xt{i}")
                   for i in range(QB)]
            wu = pp.tile([P, P], bf16, tag="wu")
            qnT = pp.tile([P, M, N], bf16, tag="qnT")
            knT = pp.tile([P, M, N], bf16, tag="knT")
            av = pp.tile([P, JC, HL, P], bf16, tag="av")
            onT = pp.tile([P, M, N], bf16, tag="onT")
            wq_sb = pp.tile([P, KC, M, P], bf16, tag="wq")
            wk_sb = pp.tile([P, KC, M, P], bf16, tag="wk")
            wv_sb = pp.tile([P, KC, M * P], bf16, tag="wv")
            wo_sb = pp.tile([P, M, KC, P], bf16, tag="wo")
            hsq_sb = pp.tile([P, M, P], bf16, tag="hsq")
            scl_sb = pp.tile([P, M], f32, tag="scl")
            zero_b = pp.tile([P, 1], f32, tag="zerob")

            # ---- input DMA: the critical path (wk then x blocks, in
            # consumption order) gets the HBM to itself on one queue; the
            # later-needed weights go on the gpsimd queue whose triggers are
            # emitted after the first K factor so they don't compete ----
            nc.sync.dma_start(out=hsq_sb[:], in_=hsq[:])
            nc.sync.dma_start(out=scl_sb[:], in_=sclq2[:])
            nc.sync.dma_start(out=wk_sb[:], in_=wkb[:])
            for i in range(QB):
                nc.sync.dma_start(out=xts[i][:], in_=xTb[:, i])

            nc.vector.memset(zero_b[:], 0.0)
            nc.vector.memset(wu[:], 0.0)
            nc.vector.memset(av[:, :, :, 0:DH], 1.0)

            # ---- PE warmup: junk matmuls during the initial DMA so the
            # clock gate is released before real work arrives; operand is a
            # memset tile so nothing waits on DMA ----
            warm = ps.tile([P, 2, NQ], f32, tag="avp", name="warm")
            for _ in range(WARMUP):
                nc.tensor.matmul(warm[:, 0, 0:P], wu[:], wu[:],
                                 start=True, stop=True)

            # ---------------- Phase A: K then Q projections + norms --------
            def factor_apply(nT, sqs_list, qb):
                qs = slice(qb * NQ, (qb + 1) * NQ)
                for m in range(M):
                    sqf = pa.tile([P, NQ], f32, tag="sqf")
                    nc.vector.reciprocal_approx_fast(
                        out=sqf[:], in_=sqs_list[m][:])
                    nc.gpsimd.tensor_mul(nT[:, m, qs], sqf[:], nT[:, m, qs])

            def emit_norm(pend_sq, q_scale):
                psq, pm, plist = pend_sq
                pn = ps.tile([P, NQ], f32, tag="avp", name="pn")
                nc.tensor.matmul(pn[:], hsq_sb[:, pm, :], psq[:],
                                 start=True, stop=True)
                sqs = pa.tile([P, NQ], f32, tag="sqs", bufs=4)
                nc.scalar.activation(
                    sqs[:], pn[:], AF.Sqrt, bias=zero_b[:],
                    scale=scl_sb[:, pm : pm + 1] if q_scale else 1.0)
                plist.append(sqs)

            def proj_side(w_sb, nT, q_scale, after_first_factor=None):
                # norm matmul for each (qb, m) emitted one step late so the
                # PE never waits in-order on the Act square chain
                pend = None
                pend_sq = None
                for qb in range(QB):
                    qs = slice(qb * NQ, (qb + 1) * NQ)
                    sqs_list = []
                    for m in range(M):
                        pq = ps.tile([P, NQ], f32, tag="sps", name="pq")
                        for kc in range(KC):
                            nc.tensor.matmul(pq[:], w_sb[:, kc, m, :],
                                             xts[qb][:, kc, :],
                                             start=(kc == 0),
                                             stop=(kc == KC - 1))
                        nc.vector.tensor_copy(nT[:, m, qs], pq[:])
                        sq = pa.tile([P, NQ], bf16, tag="sq")
                        nc.scalar.activation(sq[:], pq[:], AF.Square,
                                             bias=zero_b[:])
                        if pend_sq is not None:
                            emit_norm(pend_sq, q_scale)
                        pend_sq = (sq, m, sqs_list)
                    if pend is not None:
                        factor_apply(nT, *pend)
                        if after_first_factor is not None:
                            after_first_factor()
                            after_first_factor = None
                    pend = (sqs_list, qb)
                emit_norm(pend_sq, q_scale)
                factor_apply(nT, *pend)

            def load_late_weights():
                # gpsimd reaches these triggers only after the first factor
                # mul (~16us in), so the critical x/wk stream owns the HBM
                # until then
                nc.gpsimd.dma_start(out=wq_sb[:], in_=wqb[:])
                nc.gpsimd.dma_start(out=wv_sb[:], in_=wvb[:])
                nc.gpsimd.dma_start(out=wo_sb[:], in_=wob[:])

            proj_side(wk_sb, knT, False, load_late_weights)  # K first
            proj_side(wq_sb, qnT, True)   # Q: temp folded into sqrt scale

            # ------------- Phase B: attention (+ V weave in sweep 0) -------
            def emit_v(jc):
                pv = ps.tile([P, M * P], f32, tag="sps", name="pv")
                xj = xts[jc // 4]
                js = slice((jc % 4) * P, (jc % 4) * P + P)
                for kc in range(KC):
                    nc.tensor.matmul(pv[:], xj[:, kc, js],
                                     wv_sb[:, kc, :],
                                     start=(kc == 0), stop=(kc == KC - 1))
                nc.vector.tensor_copy(
                    av[:, jc, :, DH:P],
                    pv[:].rearrange("p (h d) -> p h d", d=DH))

            pend = []
            norm_q = []   # deferred normalize ops, drained on DVE-free slots

            def flush_pend():
                if not pend:
                    return
                pet, pkc, pavp, pqb, ppr = pend.pop(0)
                nc.tensor.matmul(pavp[:, 0, :], av[:, pkc, 2 * ppr, :],
                                 pet[:, 0, :],
                                 start=(pkc == 0), stop=(pkc == JC - 1))
                nc.tensor.matmul(pavp[:, 1, :], av[:, pkc, 2 * ppr + 1, :],
                                 pet[:, 1, :],
                                 start=(pkc == 0), stop=(pkc == JC - 1))
                if pkc == JC - 1:
                    # denominators arrive replicated on partitions 0:64 (the
                    # ones block); one fused reciprocal covers both heads.
                    # The three DVE ops are queued and drained one per
                    # Act-full slot so they never delay a DVE exp.
                    pqs = slice(pqb * NQ, (pqb + 1) * NQ)
                    st = {}

                    def op_rec(pavp=pavp, st=st):
                        rec = pa.tile([DH, 2, NQ], f32, tag="dn", name="rec")
                        nc.vector.reciprocal_approx_fast(
                            out=rec[:], in_=pavp[0:DH, :, :])
                        st["rec"] = rec

                    def op_mul0(pavp=pavp, ppr=ppr, pqs=pqs, st=st):
                        nc.vector.tensor_mul(onT[0:DH, ppr, pqs],
                                             pavp[DH:P, 0, :],
                                             st["rec"][:, 0, :])

                    def op_mul1(pavp=pavp, ppr=ppr, pqs=pqs, st=st):
                        nc.vector.tensor_mul(onT[DH:P, ppr, pqs],
                                             pavp[DH:P, 1, :],
                                             st["rec"][:, 1, :])

                    norm_q.extend([op_rec, op_mul0, op_mul1])

            for qb in range(QB):
                qs = slice(qb * NQ, (qb + 1) * NQ)
                for pr in range(M):       # head pair (2pr, 2pr+1)
                    avp = ps.tile([P, 2, NQ], f32, tag="avp", name="avp")
                    for kc in range(JC):
                        ks = slice(kc * P, (kc + 1) * P)
                        if qb == 0 and pr == 0:
                            emit_v(kc)    # weave V projection into sweep 0
                        sp = ps.tile([P, 2, NQ], f32, tag="sps", name="sp")
                        nc.tensor.matmul(sp[:, 0, :], knT[0:64, pr, ks],
                                         qnT[0:64, pr, qs],
                                         start=True, stop=True)
                        nc.tensor.matmul(sp[:, 1, :], knT[64:P, pr, ks],
                                         qnT[64:P, pr, qs],
                                         start=True, stop=True)
                        # A@V runs two kc behind (across sweep boundaries
                        # too) so the PE never waits in-order on exps still
                        # in flight on the Act/DVE queues
                        if len(pend) >= 2:
                            flush_pend()
                        et = pa.tile([P, 2, NQ], bf16, tag="et", bufs=5)
                        if kc % ACT_FULL == ACT_FULL - 1:
                            nc.scalar.activation(et[:], sp[:], AF.Exp,
                                                 bias=zero_b[:])
                            if norm_q:
                                norm_q.pop(0)()
                        else:
                            nc.scalar.activation(et[:, 0, :], sp[:, 0, :],
                                                 AF.Exp, bias=zero_b[:])
                            nc.vector.tensor_scalar(
                                out=et[:, 1, :].bitcast(i16),
                                in0=sp[:, 1, :],
                                scalar1=EXP_A16, scalar2=EXP_B16,
                                op0=OP.mult, op1=OP.add)
                        pend.append((et, kc, avp, qb, pr))
            flush_pend()
            flush_pend()
            while norm_q:
                norm_q.pop(0)()

            # ------------- Phase C: output projection ----------------------
            for qb in range(QB):
                qs = slice(qb * NQ, (qb + 1) * NQ)
                for mo in range(KC):
                    idx = qb * KC + mo
                    cp = ps.tile([P, NQ], f32,
                                 tag=("sps" if idx % 2 == 0 else "avp"),
                                 name="cp")
                    for g in range(M):
                        nc.tensor.matmul(cp[:], wo_sb[:, g, mo, :],
                                         onT[:, g, qs],
                                         start=(g == 0), stop=(g == M - 1))
                    oc = pout.tile([P, NQ], bf16, tag="ot", name="oc",
                                   bufs=6)
                    if idx % 2 == 0:
                        nc.vector.tensor_copy(oc[:], cp[:])
                    else:
                        nc.scalar.copy(oc[:], cp[:])
                    eng = nc.sync if idx % 2 == 0 else nc.gpsimd
                    eng.dma_start(out=outT[mo * P : (mo + 1) * P, qs],
                                  in_=oc[:])

    nc.compile()
    _CACHE["nc"] = nc
    return nc


def run(inputs, trace=False):
    import ml_dtypes
    from concourse.bass_utils import run_bass_kernel_spmd

    x = np.asarray(inputs["x"], np.float32)
    w_qkv = np.asarray(inputs["w_qkv"], np.float32)
    w_out = np.asarray(inputs["w_out"], np.float32)
    b_out = np.asarray(inputs["b_out"], np.float32)
    logit_scale = np.asarray(inputs["logit_scale"], np.float32)

    nc = _build()
    bf = ml_dtypes.bfloat16

    scl = np.exp(np.minimum(logit_scale.reshape(H), MAX_LOG_SCALE))

    # [P, QB, KC, NQ]: partition-major, then 512-token block, then dim chunk
    xTb = [np.ascontiguousarray(
        x[b].T.reshape(KC, P, QB, NQ).transpose(1, 2, 0, 3)).astype(bf)
        for b in range(B)]

    in_maps = []
    for c in range(8):
        b, hg = c // 4, c % 4
        cs = slice(hg * 256, (hg + 1) * 256)
        wq = np.ascontiguousarray(
            w_qkv[:, 0:INNER][:, cs].reshape(KC, P, M, P)
            .transpose(1, 0, 2, 3)).astype(bf)
        wk = np.ascontiguousarray(
            w_qkv[:, INNER:2 * INNER][:, cs].reshape(KC, P, M, P)
            .transpose(1, 0, 2, 3)).astype(bf)
        wv = np.ascontiguousarray(
            w_qkv[:, 2 * INNER:3 * INNER][:, cs].reshape(KC, P, M * P)
            .transpose(1, 0, 2)).astype(bf)
        wo = np.ascontiguousarray(
            w_out[cs, :].reshape(M, P, KC, P).transpose(1, 0, 2, 3)).astype(bf)
        hs = np.zeros((P, M, P), bf)
        hs[0:64, :, 0:64] = 1.0
        hs[64:P, :, 64:P] = 1.0
        sc2 = np.empty((P, M), np.float32)
        for m in range(M):
            sc2[0:64, m] = scl[4 * hg + 2 * m] ** -2.0
            sc2[64:P, m] = scl[4 * hg + 2 * m + 1] ** -2.0
        in_maps.append({
            "xTb": xTb[b], "wqb": wq, "wkb": wk, "wvb": wv, "wob": wo,
            "hsq": hs, "sclq2": sc2,
        })

    res = run_bass_kernel_spmd(nc, in_maps, list(range(8)), trace=trace)

    out = np.empty((B, N, DIM), np.float32)
    for b in range(B):
        acc = res.results[4 * b]["outT"].astype(np.float32)
        for hg in range(1, 4):
            acc = acc + res.results[4 * b + hg]["outT"].astype(np.float32)
        out[b] = acc.T + b_out
    return out, res


def kernel(**inputs):
    out, _ = run(inputs, trace=False)
    return out


# revision 28
# speedup vs baseline: 1.1631x; 1.0073x over previous
"""Cosine-similarity multi-head attention on 8 Trainium2 NeuronCores.

Sharding: tensor-parallel over (batch, head-group). Core c (c = b*4 + hg)
computes heads [4*hg, 4*hg+4) of batch b for ALL 2048 query tokens, then a
partial output projection over its 256 inner features.  The host sums the 4
partial outputs per batch and adds b_out (the "all-reduce" of the hint, done
during the host-side gather).  No K/V duplication: each projection row is
computed exactly once across the machine.

Per-core layouts:
  - xt   [128, 8, 2048]  x[b]^T, feature-chunked (bf16)
  - qnT/knT [128, 2, 2048] Q^T/K^T: chunk m holds local heads 2m (parts 0:64)
    and 2m+1 (parts 64:128); normalized in place (bf16)
  - av   [128, 16, 4, 128] V token-major per (key-chunk, head); cols 0:64 are
    a ones block so each A@V matmul also accumulates softmax denominators
  - softmax: no max-subtraction (|logits| <= 10, exp safe in f32)
  - exp is split across engines: Act computes exact exp; the DVE computes a
    Schraudolph bitcast exp (scale+bias -> int16 convert = bf16 bits) for a
    share of the keys.  Softmax self-normalization + averaging over many keys
    keeps the ~3% sawtooth error far below the output tolerance.
  - norm factors are broadcast across partitions with small K=128 PE matmuls
    against a 0/1 head-selector matrix (hsq) -- no DRAM round trips.
  - out-projection runs as a final phase reusing the attention PSUM rings;
    partial outputs leave as bf16 (summed in f32 on the host).
"""

import numpy as np

B, N, DIM, H, DH = 2, 2048, 1024, 16, 64
INNER = H * DH
P = 128
KC = DIM // P        # 8 contraction chunks of the model dim
JC = N // P          # 16 key-token chunks of 128
QB = 4               # query blocks of 512
NQ = N // QB         # 512
HL = 4               # heads per core
M = 2                # feature chunks per core (4 heads * 64 = 256)
MAX_LOG_SCALE = float(np.log(1.0 / 0.01))

# Schraudolph fast-exp constants (int16 convert writes bf16 bit pattern)
EXP_A16 = (2.0 ** 23 / np.log(2.0)) / 65536.0
EXP_B16 = (127.0 * 2.0 ** 23 - 360777.0) / 65536.0 + 0.5

ACT_FULL_SET = (7, 15)  # key chunks where Act computes both heads (DVE-free)
WARMUP = 34          # junk N=128 matmuls to warm the PE clock during DMA

_CACHE = {}


def _build():
    if "nc" in _CACHE:
        return _CACHE["nc"]
    import concourse.bass as bass
    import concourse.bacc as bacc
    import concourse.mybir as mybir
    import concourse.tile as tile

    f32 = mybir.dt.float32
    i16 = mybir.dt.int16
    bf16 = mybir.dt.bfloat16
    AF = mybir.ActivationFunctionType
    OP = mybir.AluOpType

    nc = bacc.Bacc("TRN2", target_bir_lowering=False)

    xTb = nc.declare_dram_parameter("xTb", [P, QB, KC, NQ], bf16,
                                    isOutput=False)
    wqb = nc.declare_dram_parameter("wqb", [P, KC, M, P], bf16, isOutput=False)
    wkb = nc.declare_dram_parameter("wkb", [P, KC, M, P], bf16, isOutput=False)
    wvb = nc.declare_dram_parameter("wvb", [P, KC, M * P], bf16, isOutput=False)
    wob = nc.declare_dram_parameter("wob", [P, M, KC, P], bf16, isOutput=False)
    hsq = nc.declare_dram_parameter("hsq", [P, M, P], bf16, isOutput=False)
    sclq2 = nc.declare_dram_parameter("sclq2", [P, M], f32, isOutput=False)
    outT = nc.declare_dram_parameter("outT", [DIM, N], bf16, isOutput=True)

    with tile.TileContext(nc) as tc:
        with (
            tc.tile_pool(name="persist", bufs=1) as pp,
            tc.tile_pool(name="work", bufs=2) as pa,
            tc.tile_pool(name="pout", bufs=4) as pout,
            tc.tile_pool(name="ps", bufs=2, space="PSUM") as ps,
        ):
            # one tile (and one DMA) per 512-token block so the first
            # projection matmul only waits on its own block's transfer
            xts = [pp.tile([P, KC, NQ], bf16, tag=f"xt{i}", name=f"xt{i}")
                   for i in range(QB)]
            wu = pp.tile([P, P], bf16, tag="wu")
            qnT = pp.tile([P, M, N], bf16, tag="qnT")
            knT = pp.tile([P, M, N], bf16, tag="knT")
            av = pp.tile([P, JC, HL, DH], bf16, tag="av")
            ones_t = pp.tile([P, DH], bf16, tag="ones")
            onT = pp.tile([P, M, N], bf16, tag="onT")
            wq_sb = pp.tile([P, KC, M, P], bf16, tag="wq")
            wk_sb = pp.tile([P, KC, M, P], bf16, tag="wk")
            wv_sb = pp.tile([P, KC, M * P], bf16, tag="wv")
            wo_sb = pp.tile([P, M, KC, P], bf16, tag="wo")
            hsq_sb = pp.tile([P, M, P], bf16, tag="hsq")
            scl_sb = pp.tile([P, M], f32, tag="scl")
            zero_b = pp.tile([P, 1], f32, tag="zerob")

            # ---- input DMA: the critical path (wk then x blocks, in
            # consumption order) gets the HBM to itself on one queue; the
            # later-needed weights go on the gpsimd queue whose triggers are
            # emitted after the first K factor so they don't compete ----
            nc.sync.dma_start(out=hsq_sb[:], in_=hsq[:])
            nc.sync.dma_start(out=scl_sb[:], in_=sclq2[:])
            nc.scalar.dma_start(out=wk_sb[:], in_=wkb[:])
            for i in range(QB):
                nc.sync.dma_start(out=xts[i][:, 0:4, :], in_=xTb[:, i, 0:4])
                nc.gpsimd.dma_start(out=xts[i][:, 4:KC, :],
                                    in_=xTb[:, i, 4:KC])

            nc.vector.memset(zero_b[:], 0.0)
            nc.vector.memset(wu[:], 0.0)
            nc.vector.memset(ones_t[:], 1.0)

            # ---- PE warmup: junk matmuls during the initial DMA so the
            # clock gate is released before real work arrives; operand is a
            # memset tile so nothing waits on DMA ----
            warm = ps.tile([P, 2, NQ], f32, tag="sps", name="warm")
            for _ in range(WARMUP):
                nc.tensor.matmul(warm[:, 0, 0:P], wu[:], wu[:],
                                 start=True, stop=True)

            # ---------------- Phase A: K then Q projections + norms --------
            def factor_apply(nT, sqs_list, qb):
                qs = slice(qb * NQ, (qb + 1) * NQ)
                for m in range(M):
                    sqf = pa.tile([P, NQ], f32, tag="sqf")
                    nc.vector.reciprocal_approx_fast(
                        out=sqf[:], in_=sqs_list[m][:])
                    nc.gpsimd.tensor_mul(nT[:, m, qs], sqf[:], nT[:, m, qs])

            def emit_norm(pend_sq, q_scale):
                psq, pm, plist = pend_sq
                pn = ps.tile([P, NQ], f32, tag="dnB", name="pn")
                nc.tensor.matmul(pn[:], hsq_sb[:, pm, :], psq[:],
                                 start=True, stop=True)
                sqs = pa.tile([P, NQ], f32, tag="sqs", bufs=4)
                nc.scalar.activation(
                    sqs[:], pn[:], AF.Sqrt, bias=zero_b[:],
                    scale=scl_sb[:, pm : pm + 1] if q_scale else 1.0)
                plist.append(sqs)

            def proj_side(w_sb, nT, q_scale, after_first_factor=None):
                # norm matmul for each (qb, m) emitted one step late so the
                # PE never waits in-order on the Act square chain
                pend = None
                pend_sq = None
                for qb in range(QB):
                    qs = slice(qb * NQ, (qb + 1) * NQ)
                    sqs_list = []
                    for m in range(M):
                        pq = ps.tile([P, NQ], f32, tag="sps", name="pq")
                        for kc in range(KC):
                            nc.tensor.matmul(pq[:], w_sb[:, kc, m, :],
                                             xts[qb][:, kc, :],
                                             start=(kc == 0),
                                             stop=(kc == KC - 1))
                        nc.vector.tensor_copy(nT[:, m, qs], pq[:])
                        sq = pa.tile([P, NQ], bf16, tag="sq")
                        nc.scalar.activation(sq[:], pq[:], AF.Square,
                                             bias=zero_b[:])
                        if pend_sq is not None:
                            emit_norm(pend_sq, q_scale)
                        pend_sq = (sq, m, sqs_list)
                    if pend is not None:
                        factor_apply(nT, *pend)
                        if after_first_factor is not None:
                            after_first_factor()
                            after_first_factor = None
                    pend = (sqs_list, qb)
                emit_norm(pend_sq, q_scale)
                factor_apply(nT, *pend)

            def load_late_weights():
                # gpsimd reaches these triggers only after the first factor
                # mul (~16us in), so the critical x/wk stream owns the HBM
                # until then
                nc.gpsimd.dma_start(out=wq_sb[:], in_=wqb[:])
                nc.gpsimd.dma_start(out=wv_sb[:], in_=wvb[:])
                nc.gpsimd.dma_start(out=wo_sb[:], in_=wob[:])

            proj_side(wk_sb, knT, False, load_late_weights)  # K first
            proj_side(wq_sb, qnT, True)   # Q: temp folded into sqrt scale

            # ------------- Phase B: attention (+ V weave in sweep 0) -------
            def emit_v(jc):
                pv = ps.tile([P, M * P], f32, tag="sps", name="pv")
                xj = xts[jc // 4]
                js = slice((jc % 4) * P, (jc % 4) * P + P)
                for kc in range(KC):
                    nc.tensor.matmul(pv[:], xj[:, kc, js],
                                     wv_sb[:, kc, :],
                                     start=(kc == 0), stop=(kc == KC - 1))
                nc.vector.tensor_copy(
                    av[:, jc], pv[:].rearrange("p (h d) -> p h d", d=DH))

            pend = []
            norm_q = []   # deferred normalize ops, drained on DVE-free slots

            def flush_pend():
                if not pend:
                    return
                pet, pkc, pavB, pdnB, pqb, ppr = pend.pop(0)
                st, sp_ = (pkc == 0), (pkc == JC - 1)
                # heads col-tiled into one bank (h0 -> parts 0:64, h1 ->
                # 64:128, concurrent); denominators likewise in a second bank
                nc.tensor.matmul(pavB[0:DH, :], av[:, pkc, 2 * ppr],
                                 pet[:, 0, :], start=st, stop=sp_)
                nc.tensor.matmul(pavB[DH:P, :], av[:, pkc, 2 * ppr + 1],
                                 pet[:, 1, :], start=st, stop=sp_)
                nc.tensor.matmul(pdnB[0:DH, :], ones_t[:],
                                 pet[:, 0, :], start=st, stop=sp_)
                nc.tensor.matmul(pdnB[DH:P, :], ones_t[:],
                                 pet[:, 1, :], start=st, stop=sp_)
                if sp_:
                    # full-width reciprocal + one full-width multiply; the
                    # two DVE ops drain one per Act-full slot so they never
                    # delay a DVE exp
                    pqs = slice(pqb * NQ, (pqb + 1) * NQ)
                    stt = {}

                    def op_rec(pdnB=pdnB, stt=stt):
                        rec = pa.tile([P, NQ], f32, tag="dn", name="rec")
                        nc.vector.reciprocal_approx_fast(
                            out=rec[:], in_=pdnB[:])
                        stt["rec"] = rec

                    def op_mul(pavB=pavB, ppr=ppr, pqs=pqs, stt=stt):
                        nc.vector.tensor_mul(onT[:, ppr, pqs], pavB[:],
                                             stt["rec"][:])

                    norm_q.extend([op_rec, op_mul])

            for qb in range(QB):
                qs = slice(qb * NQ, (qb + 1) * NQ)
                for pr in range(M):       # head pair (2pr, 2pr+1)
                    avB = ps.tile([P, NQ], f32, tag="avB", name="avB")
                    dnB = ps.tile([P, NQ], f32, tag="dnB", name="dnB")
                    for kc in range(JC):
                        ks = slice(kc * P, (kc + 1) * P)
                        if qb == 0 and pr == 0:
                            emit_v(kc)    # weave V projection into sweep 0
                        sp = ps.tile([P, 2, NQ], f32, tag="sps", name="sp")
                        nc.tensor.matmul(sp[:, 0, :], knT[0:64, pr, ks],
                                         qnT[0:64, pr, qs],
                                         start=True, stop=True)
                        nc.tensor.matmul(sp[:, 1, :], knT[64:P, pr, ks],
                                         qnT[64:P, pr, qs],
                                         start=True, stop=True)
                        # A@V runs two kc behind (across sweep boundaries
                        # too) so the PE never waits in-order on exps still
                        # in flight on the Act/DVE queues
                        if len(pend) >= 2:
                            flush_pend()
                        et = pa.tile([P, 2, NQ], bf16, tag="et", bufs=5)
                        if kc in ACT_FULL_SET:
                            nc.scalar.activation(et[:], sp[:], AF.Exp,
                                                 bias=zero_b[:])
                            if norm_q:
                                norm_q.pop(0)()
                        else:
                            nc.scalar.activation(et[:, 0, :], sp[:, 0, :],
                                                 AF.Exp, bias=zero_b[:])
                            nc.vector.tensor_scalar(
                                out=et[:, 1, :].bitcast(i16),
                                in0=sp[:, 1, :],
                                scalar1=EXP_A16, scalar2=EXP_B16,
                                op0=OP.mult, op1=OP.add)
                        pend.append((et, kc, avB, dnB, qb, pr))
            flush_pend()
            flush_pend()
            while norm_q:
                norm_q.pop(0)()

            # ------------- Phase C: output projection ----------------------
            for qb in range(QB):
                qs = slice(qb * NQ, (qb + 1) * NQ)
                for mo in range(KC):
                    idx = qb * KC + mo
                    cp = ps.tile([P, NQ], f32,
                                 tag=("avB" if idx % 2 == 0 else "dnB"),
                                 name="cp")
                    for g in range(M):
                        nc.tensor.matmul(cp[:], wo_sb[:, g, mo, :],
                                         onT[:, g, qs],
                                         start=(g == 0), stop=(g == M - 1))
                    oc = pout.tile([P, NQ], bf16, tag="ot", name="oc",
                                   bufs=6)
                    if idx % 2 == 0:
                        nc.vector.tensor_copy(oc[:], cp[:])
                    else:
                        nc.scalar.copy(oc[:], cp[:])
                    eng = nc.sync if idx % 2 == 0 else nc.gpsimd
                    eng.dma_start(out=outT[mo * P : (mo + 1) * P, qs],
                                  in_=oc[:])

    nc.compile()
    _CACHE["nc"] = nc
    return nc


def run(inputs, trace=False):
    import ml_dtypes
    from concourse.bass_utils import run_bass_kernel_spmd

    x = np.asarray(inputs["x"], np.float32)
    w_qkv = np.asarray(inputs["w_qkv"], np.float32)
    w_out = np.asarray(inputs["w_out"], np.float32)
    b_out = np.asarray(inputs["b_out"], np.float32)
    logit_scale = np.asarray(inputs["logit_scale"], np.float32)

    nc = _build()
    bf = ml_dtypes.bfloat16

    scl = np.exp(np.minimum(logit_scale.reshape(H), MAX_LOG_SCALE))

    # [P, QB, KC, NQ]: partition-major, then 512-token block, then dim chunk
    xTb = [np.ascontiguousarray(
        x[b].T.reshape(KC, P, QB, NQ).transpose(1, 2, 0, 3)).astype(bf)
        for b in range(B)]

    in_maps = []
    for c in range(8):
        b, hg = c // 4, c % 4
        cs = slice(hg * 256, (hg + 1) * 256)
        wq = np.ascontiguousarray(
            w_qkv[:, 0:INNER][:, cs].reshape(KC, P, M, P)
            .transpose(1, 0, 2, 3)).astype(bf)
        wk = np.ascontiguousarray(
            w_qkv[:, INNER:2 * INNER][:, cs].reshape(KC, P, M, P)
            .transpose(1, 0, 2, 3)).astype(bf)
        wv = np.ascontiguousarray(
            w_qkv[:, 2 * INNER:3 * INNER][:, cs].reshape(KC, P, M * P)
            .transpose(1, 0, 2)).astype(bf)
        wo = np.ascontiguousarray(
            w_out[cs, :].reshape(M, P, KC, P).transpose(1, 0, 2, 3)).astype(bf)
        hs = np.zeros((P, M, P), bf)
        hs[0:64, :, 0:64] = 1.0
        hs[64:P, :, 64:P] = 1.0
        sc2 = np.empty((P, M), np.float32)
        for m in range(M):
            sc2[0:64, m] = scl[4 * hg + 2 * m] ** -2.0
            sc2[64:P, m] = scl[4 * hg + 2 * m + 1] ** -2.0
        in_maps.append({
            "xTb": xTb[b], "wqb": wq, "wkb": wk, "wvb": wv, "wob": wo,
            "hsq": hs, "sclq2": sc2,
        })

    res = run_bass_kernel_spmd(nc, in_maps, list(range(8)), trace=trace)

    out = np.empty((B, N, DIM), np.float32)
    for b in range(B):
        acc = res.results[4 * b]["outT"].astype(np.float32)
        for hg in range(1, 4):
            acc = acc + res.results[4 * b + hg]["outT"].astype(np.float32)
        out[b] = acc.T + b_out
    return out, res


def kernel(**inputs):
    out, _ = run(inputs, trace=False)
    return out
